# revision 10
# baseline (speedup 1.0000x reference)
"""CavityLoss Trainium2 kernel (nn_CavityLoss_43722767073667).

Mathematical reduction of the reference (verified against a bit-faithful
numpy emulation incl. adversarial threshold-boundary values):

  pb = (floor(pred*255) >= 128)  <=>  (pred >= c*),  c* = f32(128/255)
  The 5^3 all-ones dilation of the binary gt is an exact integer count
  >= gt (the window contains the center voxel), so
      diff = ((gt - pb*dilate(gt)) > 0) == gt * (1 - pb)     [identity]
  Non-critical voxels contribute exactly 0 to the BCE in fp32, so
      loss = -mean( gt * [pred < c*] * ln(pred) ).

Distribution: 192^3 volume flattened into 8 equal slabs, each viewed as
[128 partitions, 6912].  Pointwise + reduction only - the dilation
cancels, so no halo exchange and no collectives; the cross-core mean is
combined on the host in f64.

v7 (evolution of v4=29.3, v5=28.3, v6=29.2, all measured on HW):
  - inputs packed bf16 on the host (3.54 MB/core; pred in bf16 perturbs
    the loss ~2e-6 relative, verified numerically; gbar = 1-gt is exact
    in bf16).  The gt multiply is eliminated by folding gbar into the
    select (ln(1) == 0):
        r = max([p >= c*], gbar, p)   # r = p on critical voxels else 1
        acc += rowsum(ln(r))          # ACT Ln with fp32 accum_out
  - the kernel is DVE-bound (measured bf16 rates: tensor_scalar 0.27
    ns/col = 4x mode, tensor_tensor 0.52 = 2x, fused STT 1.06 = 1x
    only).  Big tiles use ts(is_ge) + tt(max) + tt(max); small tiles
    the 2-op stt + tt.
  - inputs stream on TWO HWDGE rings (sync + the otherwise-idle scalar
    engine) to speed up the early ramp; one semaphore per transfer, so
    no cross-ring ordering is assumed.
  - Ln chunks are decoupled from DMA tiles (r_sb is contiguous); the
    last chunk's [128,1] column is DMA'd out by the SCALAR engine right
    after its own accumulator read (program order - no cross-engine
    hop), overlapping the sync-issued DMA of the earlier columns.
  - tile 0 tiny (128 cols) for an early DVE start; a 1-descriptor
    warmup DMA ahead of it absorbs part of the first-transfer
    queue-warmup + receipt latency (~2us).
"""

import numpy as np
import ml_dtypes

import concourse.bacc as bacc
import concourse.mybir as mybir
from concourse.bass_utils import run_bass_kernel_spmd

D = 192
N_CORES = 8
P = 128
TOTAL = D * D * D              # 7_077_888
PER_CORE = TOTAL // N_CORES    # 884_736
FREE = PER_CORE // P           # 6_912
SIZES = [128, 1280, 1536, 1408, 1280, 1024, 256]
assert sum(SIZES) == FREE
NT = len(SIZES)
THREE_OP_MIN = 556             # tiles >= this use ts+tt+tt, else stt+tt
LN_CHUNKS = [[0, 1], [2], [3], [4], [5], [6]]   # tile groups per Ln call
NCH = len(LN_CHUNKS)

C_STAR = float(np.float32(128.0) / np.float32(255.0))

_CACHE = {}


def _build():
    nc = bacc.Bacc("TRN2", name="cavity_loss")
    f32 = mybir.dt.float32
    bf16 = mybir.dt.bfloat16

    ins = [
        nc.dram_tensor(f"in{t}", [P, 2 * s], bf16, kind="ExternalInput")
        for t, s in enumerate(SIZES)
    ]
    out1 = nc.dram_tensor("out1", [P, NCH - 1], f32, kind="ExternalOutput")
    out2 = nc.dram_tensor("out2", [P, 1], f32, kind="ExternalOutput")

    ge = mybir.AluOpType.is_ge
    mx = mybir.AluOpType.max
    Ln = mybir.ActivationFunctionType.Ln

    in_sb = [
        nc.alloc_sbuf_tensor(f"in_sb{t}", [P, 2 * s], bf16).ap()
        for t, s in enumerate(SIZES)
    ]
    b_sb = nc.alloc_sbuf_tensor("b_sb", [P, FREE], bf16).ap()
    m_sb = nc.alloc_sbuf_tensor("m_sb", [P, FREE], bf16).ap()
    r_sb = nc.alloc_sbuf_tensor("r_sb", [P, FREE], bf16).ap()
    acc = nc.alloc_sbuf_tensor("acc_sb", [P, NCH], f32).ap()
    warm = nc.alloc_sbuf_tensor("warm_sb", [1, 64], bf16).ap()

    s_warm = nc.alloc_semaphore("s_warm")
    s_in = [nc.alloc_semaphore(f"s_in{t}") for t in range(NT)]
    s_r = nc.alloc_semaphore("s_r")
    s_acc = nc.alloc_semaphore("s_acc")
    s_out1 = nc.alloc_semaphore("s_out1")
    s_out2 = nc.alloc_semaphore("s_out2")

    offs = np.concatenate([[0], np.cumsum(SIZES)]).tolist()
    sls = [slice(offs[t], offs[t + 1]) for t in range(NT)]

    # two HWDGE rings: sync takes even tiles (plus the warmup), scalar odd
    nc.sync.dma_start(warm[:], ins[0][0:1, 0:64]).then_inc(s_warm, 16)
    for t in range(NT):
        eng = nc.sync if t % 2 == 0 else nc.scalar
        eng.dma_start(in_sb[t][:, :], ins[t][:, :]).then_inc(s_in[t], 16)

    # dummy Ln pulls the ACT table load into the DMA window
    dummy = nc.alloc_sbuf_tensor("dummy_sb", [P, 1], f32).ap()
    one = nc.const_aps.tensor(1.0, (P, 1))
    nc.scalar.activation(dummy[:], one, Ln)

    # vector: select per tile; s_r counts completed tiles (in tile order)
    for t in range(NT):
        s = SIZES[t]
        sl = sls[t]
        p_ap = in_sb[t][:, 0:s]
        g_ap = in_sb[t][:, s : 2 * s]
        nc.vector.wait_ge(s_in[t], 16)
        if s >= THREE_OP_MIN:
            nc.vector.tensor_scalar(b_sb[:, sl], p_ap, C_STAR, None, ge)
            nc.vector.tensor_tensor(m_sb[:, sl], b_sb[:, sl], g_ap, mx)
        else:
            nc.vector.scalar_tensor_tensor(m_sb[:, sl], p_ap, C_STAR, g_ap, ge, mx)
        nc.vector.tensor_tensor(
            r_sb[:, sl], m_sb[:, sl], p_ap, mx
        ).then_inc(s_r, 1)

    # scalar: chunked Ln with fp32 accumulate; elementwise output lands on
    # the dead b tile region (b[t] is free once m[t] is computed)
    for j, tiles in enumerate(LN_CHUNKS):
        lo, hi = offs[tiles[0]], offs[tiles[-1] + 1]
        nc.scalar.wait_ge(s_r, tiles[-1] + 1)
        nc.scalar.activation(
            b_sb[:, lo:hi], r_sb[:, lo:hi], Ln,
            accum_out=acc[:, j : j + 1],
        ).then_inc(s_acc, 1)

    # finalize: sync DMAs columns 0..NCH-2 once they are ready; scalar DMAs
    # its own last column immediately after the final accumulator read
    nc.scalar.dma_start(out2[:], acc[:, NCH - 1 : NCH]).then_inc(s_out2, 16)
    nc.sync.wait_ge(s_acc, NCH - 1)
    nc.sync.dma_start(out1[:], acc[:, 0 : NCH - 1]).then_inc(s_out1, 16)
    nc.sync.wait_ge(s_out1, 16)
    nc.sync.wait_ge(s_out2, 16)

    nc.compile()
    return nc


def _get_nc():
    if "nc" not in _CACHE:
        _CACHE["nc"] = _build()
    return _CACHE["nc"]


_OFFS = np.concatenate([[0], np.cumsum(SIZES)]).tolist()


def _shard(pred, gt):
    """Per core, per tile: one contiguous bf16 [128, 2*s] array [pred|1-gt]."""
    pf = np.asarray(pred, dtype=np.float32).reshape(-1).astype(ml_dtypes.bfloat16)
    gbar = (np.float32(1.0) - np.asarray(gt, dtype=np.float32).reshape(-1)).astype(
        ml_dtypes.bfloat16
    )
    assert pf.size == TOTAL and gbar.size == TOTAL
    in_maps = []
    for c in range(N_CORES):
        pc = pf[c * PER_CORE : (c + 1) * PER_CORE].reshape(P, FREE)
        gc = gbar[c * PER_CORE : (c + 1) * PER_CORE].reshape(P, FREE)
        m = {}
        for t in range(NT):
            sl = slice(_OFFS[t], _OFFS[t + 1])
            m[f"in{t}"] = np.ascontiguousarray(
                np.concatenate([pc[:, sl], gc[:, sl]], axis=1)
            )
        in_maps.append(m)
    return in_maps


def run_spmd(pred, gt, **kw):
    """Shard, run on 8 cores; returns BassKernelResults (kw e.g. trace=True)."""
    in_maps = _shard(pred, gt)
    return run_bass_kernel_spmd(
        _get_nc(), in_maps, core_ids=list(range(N_CORES)), **kw
    )


def kernel(pred, gt):
    res = run_spmd(pred, gt)
    total = 0.0
    for r in res.results:
        total += float(r["out1"].astype(np.float64).sum())
        total += float(r["out2"].astype(np.float64).sum())
    return np.asarray(np.float32(-total / TOTAL))


# revision 11
# speedup vs baseline: 1.0172x; 1.0172x over previous
"""CavityLoss Trainium2 kernel (nn_CavityLoss_43722767073667).

Mathematical reduction of the reference (verified against a bit-faithful
numpy emulation incl. adversarial threshold-boundary values):

  pb = (floor(pred*255) >= 128)  <=>  (pred >= c*),  c* = f32(128/255)
  The 5^3 all-ones dilation of the binary gt is an exact integer count
  >= gt (the window contains the center voxel), so
      diff = ((gt - pb*dilate(gt)) > 0) == gt * (1 - pb)     [identity]
  Non-critical voxels contribute exactly 0 to the BCE in fp32, so
      loss = -mean( gt * [pred < c*] * ln(pred) ).

Distribution: 192^3 volume flattened into 8 equal slabs, each viewed as
[128 partitions, 6912].  Pointwise + reduction only - the dilation
cancels, so no halo exchange and no collectives; the cross-core mean is
combined on the host in f64.

v8 (evolution of v4=29.3, v5=28.3, v6/v7 regressions, measured on HW):
  - inputs packed bf16 on the host (3.54 MB/core; pred in bf16 perturbs
    the loss ~2e-6 relative, verified numerically; gbar = 1-gt is exact
    in bf16).  The gt multiply is eliminated by folding gbar into the
    select (ln(1) == 0):
        r = max([p >= c*], gbar, p)   # r = p on critical voxels else 1
        acc += rowsum(ln(r))          # ACT Ln with fp32 accum_out
  - the kernel is DVE-bound (measured bf16 rates: tensor_scalar 0.27
    ns/col = 4x mode, tensor_tensor 0.52 = 2x, fused STT 1.06 = 1x
    only).  Big tiles use ts(is_ge) + tt(max) + tt(max); small tiles
    the 2-op stt + tt.
  - ONE input ring (sync).  A second ring (v7) interleaves packets and
    delays the first tile's completion - the single-ring FIFO is what
    gets tile 0 in early.  Scalar-engine DMAs also trigger a second
    1.3us ACT table load and can be compiler-hoisted past the
    accumulator write they depend on - both bitten in v7.
  - sizes ramp up then down: tiny tile 0 (DVE starts ~10us), small
    tile 1 (kills the early-arrival stall), big middle, small tail
    (short post-last-byte chain).  Ln chunks decoupled from DMA tiles.
  - finalize: split output on the sync ring - columns 0..NCH-2 issue
    during the last Ln chunk, the [128,1] last column right after the
    final accumulator read; host sums in f64.
"""

import numpy as np
import ml_dtypes

import concourse.bacc as bacc
import concourse.mybir as mybir
from concourse.bass_utils import run_bass_kernel_spmd

D = 192
N_CORES = 8
P = 128
TOTAL = D * D * D              # 7_077_888
PER_CORE = TOTAL // N_CORES    # 884_736
FREE = PER_CORE // P           # 6_912
SIZES = [128, 640, 1280, 1536, 1408, 1280, 512, 128]
assert sum(SIZES) == FREE
NT = len(SIZES)
THREE_OP_MIN = 556             # tiles >= this use ts+tt+tt, else stt+tt
LN_CHUNKS = [[0, 1], [2], [3], [4], [5], [6], [7]]   # tile groups per Ln call
NCH = len(LN_CHUNKS)

C_STAR = float(np.float32(128.0) / np.float32(255.0))

_CACHE = {}


def _build():
    nc = bacc.Bacc("TRN2", name="cavity_loss")
    f32 = mybir.dt.float32
    bf16 = mybir.dt.bfloat16

    ins = [
        nc.dram_tensor(f"in{t}", [P, 2 * s], bf16, kind="ExternalInput")
        for t, s in enumerate(SIZES)
    ]
    out1 = nc.dram_tensor("out1", [P, NCH - 1], f32, kind="ExternalOutput")
    out2 = nc.dram_tensor("out2", [P, 1], f32, kind="ExternalOutput")

    ge = mybir.AluOpType.is_ge
    mx = mybir.AluOpType.max
    Ln = mybir.ActivationFunctionType.Ln

    in_sb = [
        nc.alloc_sbuf_tensor(f"in_sb{t}", [P, 2 * s], bf16).ap()
        for t, s in enumerate(SIZES)
    ]
    b_sb = nc.alloc_sbuf_tensor("b_sb", [P, FREE], bf16).ap()
    m_sb = nc.alloc_sbuf_tensor("m_sb", [P, FREE], bf16).ap()
    r_sb = nc.alloc_sbuf_tensor("r_sb", [P, FREE], bf16).ap()
    acc = nc.alloc_sbuf_tensor("acc_sb", [P, NCH], f32).ap()
    warm = nc.alloc_sbuf_tensor("warm_sb", [1, 64], bf16).ap()

    s_warm = nc.alloc_semaphore("s_warm")
    s_in = [nc.alloc_semaphore(f"s_in{t}") for t in range(NT)]
    s_r = nc.alloc_semaphore("s_r")
    s_acc = nc.alloc_semaphore("s_acc")
    s_out1 = nc.alloc_semaphore("s_out1")
    s_out2 = nc.alloc_semaphore("s_out2")

    offs = np.concatenate([[0], np.cumsum(SIZES)]).tolist()
    sls = [slice(offs[t], offs[t + 1]) for t in range(NT)]

    # sync: 1-descriptor warmup absorbs queue-warmup latency, then stream
    # all merged [pred | gbar] tiles on one HWDGE ring
    nc.sync.dma_start(warm[:], ins[0][0:1, 0:64]).then_inc(s_warm, 16)
    for t in range(NT):
        nc.sync.dma_start(in_sb[t][:, :], ins[t][:, :]).then_inc(s_in[t], 16)

    # dummy Ln pulls the ACT table load into the DMA window
    dummy = nc.alloc_sbuf_tensor("dummy_sb", [P, 1], f32).ap()
    one = nc.const_aps.tensor(1.0, (P, 1))
    nc.scalar.activation(dummy[:], one, Ln)

    # vector: select per tile; s_r counts completed tiles (in tile order)
    for t in range(NT):
        s = SIZES[t]
        sl = sls[t]
        p_ap = in_sb[t][:, 0:s]
        g_ap = in_sb[t][:, s : 2 * s]
        nc.vector.wait_ge(s_in[t], 16)
        if s >= THREE_OP_MIN:
            nc.vector.tensor_scalar(b_sb[:, sl], p_ap, C_STAR, None, ge)
            nc.vector.tensor_tensor(m_sb[:, sl], b_sb[:, sl], g_ap, mx)
        else:
            nc.vector.scalar_tensor_tensor(m_sb[:, sl], p_ap, C_STAR, g_ap, ge, mx)
        nc.vector.tensor_tensor(
            r_sb[:, sl], m_sb[:, sl], p_ap, mx
        ).then_inc(s_r, 1)

    # scalar: chunked Ln with fp32 accumulate; elementwise output lands on
    # the dead b tile region (b[t] is free once m[t] is computed)
    for j, tiles in enumerate(LN_CHUNKS):
        lo, hi = offs[tiles[0]], offs[tiles[-1] + 1]
        nc.scalar.wait_ge(s_r, tiles[-1] + 1)
        nc.scalar.activation(
            b_sb[:, lo:hi], r_sb[:, lo:hi], Ln,
            accum_out=acc[:, j : j + 1],
        ).then_inc(s_acc, 1)

    # finalize on the sync ring: the big part issues while the last Ln
    # chunk runs; the [128,1] last column goes right after the final RA
    nc.sync.wait_ge(s_acc, NCH - 1)
    nc.sync.dma_start(out1[:], acc[:, 0 : NCH - 1]).then_inc(s_out1, 16)
    nc.sync.wait_ge(s_acc, NCH)
    nc.sync.dma_start(out2[:], acc[:, NCH - 1 : NCH]).then_inc(s_out2, 16)
    nc.sync.wait_ge(s_out1, 16)
    nc.sync.wait_ge(s_out2, 16)

    nc.compile()
    return nc


def _get_nc():
    if "nc" not in _CACHE:
        _CACHE["nc"] = _build()
    return _CACHE["nc"]


_OFFS = np.concatenate([[0], np.cumsum(SIZES)]).tolist()


def _shard(pred, gt):
    """Per core, per tile: one contiguous bf16 [128, 2*s] array [pred|1-gt]."""
    pf = np.asarray(pred, dtype=np.float32).reshape(-1).astype(ml_dtypes.bfloat16)
    gbar = (np.float32(1.0) - np.asarray(gt, dtype=np.float32).reshape(-1)).astype(
        ml_dtypes.bfloat16
    )
    assert pf.size == TOTAL and gbar.size == TOTAL
    in_maps = []
    for c in range(N_CORES):
        pc = pf[c * PER_CORE : (c + 1) * PER_CORE].reshape(P, FREE)
        gc = gbar[c * PER_CORE : (c + 1) * PER_CORE].reshape(P, FREE)
        m = {}
        for t in range(NT):
            sl = slice(_OFFS[t], _OFFS[t + 1])
            m[f"in{t}"] = np.ascontiguousarray(
                np.concatenate([pc[:, sl], gc[:, sl]], axis=1)
            )
        in_maps.append(m)
    return in_maps


def run_spmd(pred, gt, **kw):
    """Shard, run on 8 cores; returns BassKernelResults (kw e.g. trace=True)."""
    in_maps = _shard(pred, gt)
    return run_bass_kernel_spmd(
        _get_nc(), in_maps, core_ids=list(range(N_CORES)), **kw
    )


def kernel(pred, gt):
    res = run_spmd(pred, gt)
    total = 0.0
    for r in res.results:
        total += float(r["out1"].astype(np.float64).sum())
        total += float(r["out2"].astype(np.float64).sum())
    return np.asarray(np.float32(-total / TOTAL))


# revision 12
# speedup vs baseline: 1.2701x; 1.2486x over previous
"""CavityLoss Trainium2 kernel (nn_CavityLoss_43722767073667).

Mathematical reduction of the reference (verified against a bit-faithful
numpy emulation incl. adversarial threshold-boundary values):

  pb = (floor(pred*255) >= 128)  <=>  (pred >= c*),  c* = f32(128/255)
  The 5^3 all-ones dilation of the binary gt is an exact integer count
  >= gt (the window contains the center voxel), so
      diff = ((gt - pb*dilate(gt)) > 0) == gt * (1 - pb)     [identity]
  Non-critical voxels contribute exactly 0 to the BCE in fp32, so
      loss = -mean( gt * [pred < c*] * ln(pred) ).

Distribution: 192^3 volume flattened into 8 equal slabs, each viewed as
[128 partitions, 6912].  Pointwise + reduction only - the dilation
cancels, so no halo exchange and no collectives; the cross-core mean is
combined on the host in f64.

v8 (evolution of v4=29.3, v5=28.3, v6/v7 regressions, measured on HW):
  - inputs packed bf16 on the host (3.54 MB/core; pred in bf16 perturbs
    the loss ~2e-6 relative, verified numerically; gbar = 1-gt is exact
    in bf16).  The gt multiply is eliminated by folding gbar into the
    select (ln(1) == 0):
        r = max([p >= c*], gbar, p)   # r = p on critical voxels else 1
        acc += rowsum(ln(r))          # ACT Ln with fp32 accum_out
  - the kernel is DVE-bound (measured bf16 rates: tensor_scalar 0.27
    ns/col = 4x mode, tensor_tensor 0.52 = 2x, fused STT 1.06 = 1x
    only).  Big tiles use ts(is_ge) + tt(max) + tt(max); small tiles
    the 2-op stt + tt.
  - ONE input ring (sync).  A second ring (v7) interleaves packets and
    delays the first tile's completion - the single-ring FIFO is what
    gets tile 0 in early.  Scalar-engine DMAs also trigger a second
    1.3us ACT table load and can be compiler-hoisted past the
    accumulator write they depend on - both bitten in v7.
  - sizes ramp up then down: tiny tile 0 (DVE starts ~10us), small
    tile 1 (kills the early-arrival stall), big middle, small tail
    (short post-last-byte chain).  Ln chunks decoupled from DMA tiles.
  - finalize: split output on the sync ring - columns 0..NCH-2 issue
    during the last Ln chunk, the [128,1] last column right after the
    final accumulator read; host sums in f64.
"""

import numpy as np
import ml_dtypes

import concourse.bacc as bacc
import concourse.mybir as mybir
from concourse.bass_utils import run_bass_kernel_spmd

D = 192
N_CORES = 8
P = 128
TOTAL = D * D * D              # 7_077_888
PER_CORE = TOTAL // N_CORES    # 884_736
FREE = PER_CORE // P           # 6_912
SIZES = [128, 640, 1408, 1536, 1536, 1024, 512, 128]
assert sum(SIZES) == FREE
NT = len(SIZES)
THREE_OP_MIN = 556             # tiles >= this use ts+tt+tt, else stt+tt
LN_CHUNKS = [[0, 1], [2], [3], [4], [5], [6], [7]]   # tile groups per Ln call
NCH = len(LN_CHUNKS)

C_STAR = float(np.float32(128.0) / np.float32(255.0))

_CACHE = {}


def _build():
    nc = bacc.Bacc("TRN2", name="cavity_loss")
    f32 = mybir.dt.float32
    bf16 = mybir.dt.bfloat16

    ins = [
        nc.dram_tensor(f"in{t}", [P, 2 * s], bf16, kind="ExternalInput")
        for t, s in enumerate(SIZES)
    ]
    out = nc.dram_tensor("out", [P, NCH], f32, kind="ExternalOutput")

    ge = mybir.AluOpType.is_ge
    mx = mybir.AluOpType.max
    Ln = mybir.ActivationFunctionType.Ln

    in_sb = [
        nc.alloc_sbuf_tensor(f"in_sb{t}", [P, 2 * s], bf16).ap()
        for t, s in enumerate(SIZES)
    ]
    b_sb = nc.alloc_sbuf_tensor("b_sb", [P, FREE], bf16).ap()
    m_sb = nc.alloc_sbuf_tensor("m_sb", [P, FREE], bf16).ap()
    r_sb = nc.alloc_sbuf_tensor("r_sb", [P, FREE], bf16).ap()
    acc = nc.alloc_sbuf_tensor("acc_sb", [P, NCH], f32).ap()
    warm = nc.alloc_sbuf_tensor("warm_sb", [1, 64], bf16).ap()

    s_warm = nc.alloc_semaphore("s_warm")
    s_in = [nc.alloc_semaphore(f"s_in{t}") for t in range(NT)]
    s_r = nc.alloc_semaphore("s_r")
    s_acc = nc.alloc_semaphore("s_acc")
    s_out = nc.alloc_semaphore("s_out")

    offs = np.concatenate([[0], np.cumsum(SIZES)]).tolist()
    sls = [slice(offs[t], offs[t + 1]) for t in range(NT)]

    # sync: 1-descriptor warmup absorbs queue-warmup latency, then stream
    # all merged [pred | gbar] tiles on one HWDGE ring
    nc.sync.dma_start(warm[:], ins[0][0:1, 0:64]).then_inc(s_warm, 16)
    for t in range(NT):
        nc.sync.dma_start(in_sb[t][:, :], ins[t][:, :]).then_inc(s_in[t], 16)

    # dummy Ln pulls the ACT table load into the DMA window
    dummy = nc.alloc_sbuf_tensor("dummy_sb", [P, 1], f32).ap()
    one = nc.const_aps.tensor(1.0, (P, 1))
    nc.scalar.activation(dummy[:], one, Ln)

    # vector: select per tile; s_r counts completed tiles (in tile order)
    for t in range(NT):
        s = SIZES[t]
        sl = sls[t]
        p_ap = in_sb[t][:, 0:s]
        g_ap = in_sb[t][:, s : 2 * s]
        nc.vector.wait_ge(s_in[t], 16)
        if s >= THREE_OP_MIN:
            nc.vector.tensor_scalar(b_sb[:, sl], p_ap, C_STAR, None, ge)
            nc.vector.tensor_tensor(m_sb[:, sl], b_sb[:, sl], g_ap, mx)
        else:
            nc.vector.scalar_tensor_tensor(m_sb[:, sl], p_ap, C_STAR, g_ap, ge, mx)
        nc.vector.tensor_tensor(
            r_sb[:, sl], m_sb[:, sl], p_ap, mx
        ).then_inc(s_r, 1)

    # scalar: chunked Ln with fp32 accumulate; elementwise output lands on
    # the dead b tile region (b[t] is free once m[t] is computed)
    for j, tiles in enumerate(LN_CHUNKS):
        lo, hi = offs[tiles[0]], offs[tiles[-1] + 1]
        nc.scalar.wait_ge(s_r, tiles[-1] + 1)
        nc.scalar.activation(
            b_sb[:, lo:hi], r_sb[:, lo:hi], Ln,
            accum_out=acc[:, j : j + 1],
        ).then_inc(s_acc, 1)

    # finalize: ONE [128, NCH] out DMA.  (Splitting off a [128,1] column
    # makes 4-byte descriptors -> HBM read-modify-write, ~8us receipt.)
    nc.sync.wait_ge(s_acc, NCH)
    nc.sync.dma_start(out[:], acc[:]).then_inc(s_out, 16)
    nc.sync.wait_ge(s_out, 16)

    nc.compile()
    return nc


def _get_nc():
    if "nc" not in _CACHE:
        _CACHE["nc"] = _build()
    return _CACHE["nc"]


_OFFS = np.concatenate([[0], np.cumsum(SIZES)]).tolist()


def _shard(pred, gt):
    """Per core, per tile: one contiguous bf16 [128, 2*s] array [pred|1-gt]."""
    pf = np.asarray(pred, dtype=np.float32).reshape(-1).astype(ml_dtypes.bfloat16)
    gbar = (np.float32(1.0) - np.asarray(gt, dtype=np.float32).reshape(-1)).astype(
        ml_dtypes.bfloat16
    )
    assert pf.size == TOTAL and gbar.size == TOTAL
    in_maps = []
    for c in range(N_CORES):
        pc = pf[c * PER_CORE : (c + 1) * PER_CORE].reshape(P, FREE)
        gc = gbar[c * PER_CORE : (c + 1) * PER_CORE].reshape(P, FREE)
        m = {}
        for t in range(NT):
            sl = slice(_OFFS[t], _OFFS[t + 1])
            m[f"in{t}"] = np.ascontiguousarray(
                np.concatenate([pc[:, sl], gc[:, sl]], axis=1)
            )
        in_maps.append(m)
    return in_maps


def run_spmd(pred, gt, **kw):
    """Shard, run on 8 cores; returns BassKernelResults (kw e.g. trace=True)."""
    in_maps = _shard(pred, gt)
    return run_bass_kernel_spmd(
        _get_nc(), in_maps, core_ids=list(range(N_CORES)), **kw
    )


def kernel(pred, gt):
    res = run_spmd(pred, gt)
    total = 0.0
    for r in res.results:
        total += float(r["out"].astype(np.float64).sum())
    return np.asarray(np.float32(-total / TOTAL))


# revision 13
# speedup vs baseline: 1.2810x; 1.0086x over previous
"""CavityLoss Trainium2 kernel (nn_CavityLoss_43722767073667).

Mathematical reduction of the reference (verified against a bit-faithful
numpy emulation incl. adversarial threshold-boundary values):

  pb = (floor(pred*255) >= 128)  <=>  (pred >= c*),  c* = f32(128/255)
  The 5^3 all-ones dilation of the binary gt is an exact integer count
  >= gt (the window contains the center voxel), so
      diff = ((gt - pb*dilate(gt)) > 0) == gt * (1 - pb)     [identity]
  Non-critical voxels contribute exactly 0 to the BCE in fp32, so
      loss = -mean( gt * [pred < c*] * ln(pred) ).

Distribution: 192^3 volume flattened into 8 equal slabs, each viewed as
[128 partitions, 6912].  Pointwise + reduction only - the dilation
cancels, so no halo exchange and no collectives; the cross-core mean is
combined on the host in f64.

v8 (evolution of v4=29.3, v5=28.3, v6/v7 regressions, measured on HW):
  - inputs packed bf16 on the host (3.54 MB/core; pred in bf16 perturbs
    the loss ~2e-6 relative, verified numerically; gbar = 1-gt is exact
    in bf16).  The gt multiply is eliminated by folding gbar into the
    select (ln(1) == 0):
        r = max([p >= c*], gbar, p)   # r = p on critical voxels else 1
        acc += rowsum(ln(r))          # ACT Ln with fp32 accum_out
  - the kernel is DVE-bound (measured bf16 rates: tensor_scalar 0.27
    ns/col = 4x mode, tensor_tensor 0.52 = 2x, fused STT 1.06 = 1x
    only).  Big tiles use ts(is_ge) + tt(max) + tt(max); small tiles
    the 2-op stt + tt.
  - ONE input ring (sync).  A second ring (v7) interleaves packets and
    delays the first tile's completion - the single-ring FIFO is what
    gets tile 0 in early.  Scalar-engine DMAs also trigger a second
    1.3us ACT table load and can be compiler-hoisted past the
    accumulator write they depend on - both bitten in v7.
  - sizes ramp up then down: tiny tile 0 (DVE starts ~10us), small
    tile 1 (kills the early-arrival stall), big middle, small tail
    (short post-last-byte chain).  Ln chunks decoupled from DMA tiles.
  - finalize: split output on the sync ring - columns 0..NCH-2 issue
    during the last Ln chunk, the [128,1] last column right after the
    final accumulator read; host sums in f64.
"""

import numpy as np
import ml_dtypes

import concourse.bacc as bacc
import concourse.mybir as mybir
from concourse.bass_utils import run_bass_kernel_spmd

D = 192
N_CORES = 8
P = 128
TOTAL = D * D * D              # 7_077_888
PER_CORE = TOTAL // N_CORES    # 884_736
FREE = PER_CORE // P           # 6_912
SIZES = [128, 640, 1536, 1536, 1536, 768, 384, 256, 128]
assert sum(SIZES) == FREE
NT = len(SIZES)
THREE_OP_MIN = 556             # tiles >= this use ts+tt+tt, else stt+tt
LN_CHUNKS = [[0, 1], [2], [3], [4], [5], [6], [7], [8]]   # tile groups per Ln call
NCH = len(LN_CHUNKS)

C_STAR = float(np.float32(128.0) / np.float32(255.0))

_CACHE = {}


def _build():
    nc = bacc.Bacc("TRN2", name="cavity_loss")
    f32 = mybir.dt.float32
    bf16 = mybir.dt.bfloat16

    ins = [
        nc.dram_tensor(f"in{t}", [P, 2 * s], bf16, kind="ExternalInput")
        for t, s in enumerate(SIZES)
    ]
    out = nc.dram_tensor("out", [P, NCH], f32, kind="ExternalOutput")

    ge = mybir.AluOpType.is_ge
    mx = mybir.AluOpType.max
    Ln = mybir.ActivationFunctionType.Ln

    in_sb = [
        nc.alloc_sbuf_tensor(f"in_sb{t}", [P, 2 * s], bf16).ap()
        for t, s in enumerate(SIZES)
    ]
    b_sb = nc.alloc_sbuf_tensor("b_sb", [P, FREE], bf16).ap()
    m_sb = nc.alloc_sbuf_tensor("m_sb", [P, FREE], bf16).ap()
    r_sb = nc.alloc_sbuf_tensor("r_sb", [P, FREE], bf16).ap()
    acc = nc.alloc_sbuf_tensor("acc_sb", [P, NCH], f32).ap()
    warm = nc.alloc_sbuf_tensor("warm_sb", [1, 64], bf16).ap()

    s_warm = nc.alloc_semaphore("s_warm")
    s_in = [nc.alloc_semaphore(f"s_in{t}") for t in range(NT)]
    s_r = nc.alloc_semaphore("s_r")
    s_acc = nc.alloc_semaphore("s_acc")
    s_out = nc.alloc_semaphore("s_out")

    offs = np.concatenate([[0], np.cumsum(SIZES)]).tolist()
    sls = [slice(offs[t], offs[t + 1]) for t in range(NT)]

    # sync: 1-descriptor warmup absorbs queue-warmup latency, then stream
    # all merged [pred | gbar] tiles on one HWDGE ring
    nc.sync.dma_start(warm[:], ins[0][0:1, 0:64]).then_inc(s_warm, 16)
    for t in range(NT):
        nc.sync.dma_start(in_sb[t][:, :], ins[t][:, :]).then_inc(s_in[t], 16)

    # dummy Ln pulls the ACT table load into the DMA window
    dummy = nc.alloc_sbuf_tensor("dummy_sb", [P, 1], f32).ap()
    one = nc.const_aps.tensor(1.0, (P, 1))
    nc.scalar.activation(dummy[:], one, Ln)

    # vector: select per tile; s_r counts completed tiles (in tile order)
    for t in range(NT):
        s = SIZES[t]
        sl = sls[t]
        p_ap = in_sb[t][:, 0:s]
        g_ap = in_sb[t][:, s : 2 * s]
        nc.vector.wait_ge(s_in[t], 16)
        if s >= THREE_OP_MIN:
            nc.vector.tensor_scalar(b_sb[:, sl], p_ap, C_STAR, None, ge)
            nc.vector.tensor_tensor(m_sb[:, sl], b_sb[:, sl], g_ap, mx)
        else:
            nc.vector.scalar_tensor_tensor(m_sb[:, sl], p_ap, C_STAR, g_ap, ge, mx)
        nc.vector.tensor_tensor(
            r_sb[:, sl], m_sb[:, sl], p_ap, mx
        ).then_inc(s_r, 1)

    # scalar: chunked Ln with fp32 accumulate; elementwise output lands on
    # the dead b tile region (b[t] is free once m[t] is computed)
    for j, tiles in enumerate(LN_CHUNKS):
        lo, hi = offs[tiles[0]], offs[tiles[-1] + 1]
        nc.scalar.wait_ge(s_r, tiles[-1] + 1)
        nc.scalar.activation(
            b_sb[:, lo:hi], r_sb[:, lo:hi], Ln,
            accum_out=acc[:, j : j + 1],
        ).then_inc(s_acc, 1)

    # finalize: ONE [128, NCH] out DMA.  (Splitting off a [128,1] column
    # makes 4-byte descriptors -> HBM read-modify-write, ~8us receipt.)
    nc.sync.wait_ge(s_acc, NCH)
    nc.sync.dma_start(out[:], acc[:]).then_inc(s_out, 16)
    nc.sync.wait_ge(s_out, 16)

    nc.compile()
    return nc


def _get_nc():
    if "nc" not in _CACHE:
        _CACHE["nc"] = _build()
    return _CACHE["nc"]


_OFFS = np.concatenate([[0], np.cumsum(SIZES)]).tolist()


def _shard(pred, gt):
    """Per core, per tile: one contiguous bf16 [128, 2*s] array [pred|1-gt]."""
    pf = np.asarray(pred, dtype=np.float32).reshape(-1).astype(ml_dtypes.bfloat16)
    gbar = (np.float32(1.0) - np.asarray(gt, dtype=np.float32).reshape(-1)).astype(
        ml_dtypes.bfloat16
    )
    assert pf.size == TOTAL and gbar.size == TOTAL
    in_maps = []
    for c in range(N_CORES):
        pc = pf[c * PER_CORE : (c + 1) * PER_CORE].reshape(P, FREE)
        gc = gbar[c * PER_CORE : (c + 1) * PER_CORE].reshape(P, FREE)
        m = {}
        for t in range(NT):
            sl = slice(_OFFS[t], _OFFS[t + 1])
            m[f"in{t}"] = np.ascontiguousarray(
                np.concatenate([pc[:, sl], gc[:, sl]], axis=1)
            )
        in_maps.append(m)
    return in_maps


def run_spmd(pred, gt, **kw):
    """Shard, run on 8 cores; returns BassKernelResults (kw e.g. trace=True)."""
    in_maps = _shard(pred, gt)
    return run_bass_kernel_spmd(
        _get_nc(), in_maps, core_ids=list(range(N_CORES)), **kw
    )


def kernel(pred, gt):
    res = run_spmd(pred, gt)
    total = 0.0
    for r in res.results:
        total += float(r["out"].astype(np.float64).sum())
    return np.asarray(np.float32(-total / TOTAL))
